# revision 35
# baseline (speedup 1.0000x reference)
"""Distributed causal multi-head attention kernel for 8 TRN2 NeuronCores.

Sharding: 8 cores = 2 (batch) x 4 (head groups of 3 heads each).
Per core: qkv projection for its 3 heads (bf16 matmuls, f32 accum),
flash-style causal attention entirely in SBUF (S^T layout, no max
subtraction -- logits are bounded ~8 for this distribution), then the
output projection contribution of this core's 192 channels, summed
across the 4 cores of each batch group with per-q-macro-pair
ReduceScatters that overlap the attention loop.

Software pipeline per 512-row q-macro step tm:
  - attention for q-macro tm (S matmuls -> EXP on ACT engine -> PV),
  - QKV production for macro tm+1 and projection of macro tm-1 are
    emitted in pairs between attention kc blocks so the PE fills the
    gaps while the ACT engine runs EXPs,
  - PSUM: 2x[128,1536] rotating S tiles + 1x[128,1024] O tile = 8 banks;
    interleaved pieces borrow S-ring slots in pairs to keep the S tiles
    alternating banks.
"""

import os
import sys
import types
import ctypes
import contextlib

sys.path.insert(0, "/opt/trn_rl_repo")

import numpy as np
import ml_dtypes

import concourse.bass as bass
import concourse.mybir as mybir
import concourse.tile as tile
from concourse.masks import make_identity
from concourse import bass_utils
from concourse.bass_utils import run_bass_kernel_spmd


def _install_ntff_hook():
    """Provide antenv.axon_hooks + the ctypes NTFF profile hook so
    run_bass_kernel_spmd(trace=True) can capture HW exec times under
    axon. No-op if already present or the .so lacks the symbols."""
    try:
        from antenv.axon_hooks import get_axon_ntff_profile_hook  # noqa

        return
    except ImportError:
        pass
    try:
        import antenv
    except ImportError:
        antenv = types.ModuleType("antenv")
        sys.modules["antenv"] = antenv
    mod = types.ModuleType("antenv.axon_hooks")
    mod._hook = None
    mod.set_axon_ntff_profile_hook = lambda h: setattr(mod, "_hook", h)
    mod.get_axon_ntff_profile_hook = lambda: mod._hook
    sys.modules["antenv.axon_hooks"] = mod
    antenv.axon_hooks = mod

    so_path = "/opt/axon/libaxon_pjrt.so"
    if not os.path.exists(so_path):
        return
    try:
        lib = ctypes.CDLL(so_path)
    except OSError:
        return
    if not hasattr(lib, "axon_start_nrt_profile"):
        return
    lib.axon_start_nrt_profile.argtypes = [
        ctypes.POINTER(ctypes.c_int64),
        ctypes.c_size_t,
    ]
    lib.axon_start_nrt_profile.restype = ctypes.c_int64
    lib.axon_stop_nrt_profile.argtypes = [ctypes.c_char_p]
    lib.axon_stop_nrt_profile.restype = ctypes.c_int64

    @contextlib.contextmanager
    def _hook(output_dir, device_ids):
        import jax

        jax.devices()
        if device_ids:
            ids = (ctypes.c_int64 * len(device_ids))(*device_ids)
            rc = lib.axon_start_nrt_profile(ids, len(device_ids))
        else:
            rc = lib.axon_start_nrt_profile(None, 0)
        if rc != 0:
            raise RuntimeError(f"axon_start_nrt_profile rc={rc}")
        try:
            yield
        finally:
            n = lib.axon_stop_nrt_profile(str(output_dir).encode())
            print(f"ntff profile: {n} file(s) written to {output_dir}")

    mod._hook = _hook


# Artifact upload needs a remote bucket; keep everything local instead.
bass_utils.upload_artifacts = lambda tmpdir: str(tmpdir)

dt = mybir.dt
BF = dt.bfloat16
F32 = dt.float32

B, T, D, H, DH = 2, 4096, 768, 12, 64
NH = 3            # heads per core
GROUPS = 4        # head groups (tensor-parallel)
NDC = D // 128    # 6 contraction chunks
NTM = T // 512    # 8 t-macros
NTT = T // 128    # 32 t-tiles
CW = NH * DH      # 192 channels per core

# ReduceScatter segments: (first qm, #qms). The last segment is a
# single macro so the final (non-overlappable) collective is small;
# the others are kept small too so each transfer's DMA-bandwidth
# contention with the compute loop stays short.
RS_SEGS = [(0, 3), (3, 2), (5, 2), (7, 1)]
# output-row offset of each segment's slice in the per-core `out`
RS_OFFS = [0]
for _, _n in RS_SEGS[:-1]:
    RS_OFFS.append(RS_OFFS[-1] + 128 * _n)

_CACHE = {}


def _ocol(m):
    # O-block m (m = 4*h + qs) at col 65*m, with a bank-boundary fix:
    # blocks 0-6 in PSUM bank 0 ([0,512)), blocks 7-11 in bank 1.
    return 65 * m if m < 7 else 512 + 65 * (m - 7)


def legalize_waits(nc):
    """Walrus in this toolchain accepts at most one sync-wait per
    instruction (and none on collectives); hoist excess waits onto
    preceding same-engine NoOps."""
    wi = 0
    for f in nc.m.functions:
        for bb in f.blocks:
            new_insts = []
            changed = False
            for ins in bb.instructions:
                si = ins.sync_info
                if si is None or not si.on_wait:
                    new_insts.append(ins)
                    continue
                merged = {}
                for w in si.on_wait:
                    key = (w.sync_type, w.id, w.wait_mode, str(w.wait_reg))
                    if key not in merged or (w.wait_value or 0) > (
                        merged[key].wait_value or 0
                    ):
                        merged[key] = w
                waits = list(merged.values())
                cap = 0 if isinstance(ins, mybir.InstCollectiveCompute) else 1
                if len(waits) <= cap and len(waits) == len(si.on_wait):
                    new_insts.append(ins)
                    continue
                n_hoist = max(0, len(waits) - cap)
                hoist, keep = waits[:n_hoist], waits[n_hoist:]
                for w in hoist:
                    wi += 1
                    nop = mybir.InstNoOp(name=f"lgw_{wi}", engine=ins.engine)
                    nop.sync_info = mybir.SyncInfo(on_wait=[w], on_update=[])
                    new_insts.append(nop)
                    changed = True
                ins.sync_info = mybir.SyncInfo(
                    on_wait=keep, on_update=list(si.on_update)
                )
                new_insts.append(ins)
            if changed:
                bb.instructions = new_insts


def _build(with_bias=True):
    nc = bass.Bass()
    xT = nc.declare_dram_parameter("xT", [D, T], BF, isOutput=False)
    wqk = nc.declare_dram_parameter("wqk", [D, 2 * CW], BF, isOutput=False)
    wv = nc.declare_dram_parameter("wv", [D, CW], BF, isOutput=False)
    bqk = nc.declare_dram_parameter("bqk", [1, 2 * CW], BF, isOutput=False)
    bv = nc.declare_dram_parameter("bv", [1, CW], BF, isOutput=False)
    wp = nc.declare_dram_parameter("wp", [128, 2, D], BF, isOutput=False)
    maskp = nc.declare_dram_parameter("maskp", [128, 128], BF, isOutput=False)
    out = nc.declare_dram_parameter("out", [1024, D], BF, isOutput=True)

    rs_in = [
        nc.dram_tensor(f"rs_in{j}", [512 * n, D], BF)
        for j, (_, n) in enumerate(RS_SEGS)
    ]
    rs_out = [
        nc.dram_tensor(f"rs_out{j}", [128 * n, D], BF)
        for j, (_, n) in enumerate(RS_SEGS)
    ]
    EXP = mybir.ActivationFunctionType.Exp

    with tile.TileContext(nc) as tc:
        with (
            tc.tile_pool(name="const", bufs=1) as cpool,
            tc.tile_pool(name="work", bufs=3) as wpool,
            tc.tile_pool(name="pwork", bufs=6) as ppool,
            tc.tile_pool(name="small", bufs=2) as spool,
            tc.tile_pool(name="psS", bufs=2, space="PSUM") as pps,
            tc.tile_pool(name="psO", bufs=1, space="PSUM") as ppo,
        ):
            wqk_sb = cpool.tile([128, NDC, 2 * CW], BF)
            wv_sb = cpool.tile([128, NDC, CW], BF)
            wp_sb = cpool.tile([128, 2, D], BF)
            bqk_sb = cpool.tile([1, 2 * CW], BF)
            bv_sb = cpool.tile([1, CW], BF)
            mask_sb = cpool.tile([128, 128], BF)
            ident_sb = cpool.tile([128, 128], BF)
            ones_sb = cpool.tile([1, 512], BF)
            qkT = [
                cpool.tile([128, T], BF, name=f"qkT{m}", tag=f"qkT{m}")
                for m in range(3)
            ]
            K01 = cpool.tile([128, T], BF)   # rows 0:64 = k0, 64:128 = k1
            K2 = cpool.tile([64, T], BF)     # rows 0:64 = k2
            V_sb = cpool.tile([128, NTT, 3 * 65], BF)
            xT_sb = cpool.tile([128, NDC, T], BF)
            # normalized attention rows for q-macro tm live in slot tm%2
            attn_sb = cpool.tile([128, 2, 4, CW], BF)
            # transposed attention (proj stationary): [part, slot, chunk, t]
            # partition 64 of chunk 1 is a constant 1.0 row (bias trick).
            attnT_sb = cpool.tile([128, 2, 2, 512], BF)

            xT_v = xT[:].rearrange("(dc p) t -> p dc t", p=128)
            wqk_v = wqk[:].rearrange("(dc p) c -> p dc c", p=128)
            wv_v = wv[:].rearrange("(dc p) c -> p dc c", p=128)

            # ---- prologue: first-needed data first, split for parallel
            # DMA queues: B(0) needs xT chunk 0 + wqk (+ bqk), C(0) adds wv.
            for dc in range(NDC):
                nc.sync.dma_start(xT_sb[:, dc, 0:512], xT_v[:, dc, 0:512])
                nc.sync.dma_start(wqk_sb[:, dc, :], wqk_v[:, dc, :])
            if with_bias:
                nc.sync.dma_start(bqk_sb[:], bqk[:])
                nc.sync.dma_start(bv_sb[:], bv[:])
            for dc in range(NDC):
                nc.sync.dma_start(wv_sb[:, dc, :], wv_v[:, dc, :])
            nc.sync.dma_start(mask_sb[:], maskp[:])
            make_identity(nc, ident_sb[:])
            nc.gpsimd.memset(ones_sb[:], 1.0)
            for h in range(3):
                nc.gpsimd.memset(V_sb[:, :, 64 + 65 * h : 65 + 65 * h], 1.0)
            nc.gpsimd.memset(attnT_sb[64:65, :, 1, :], 1.0)
            nc.sync.dma_start(wp_sb[:], wp[:])

            # ---- piece emitters ----
            def emit_qk_mtile(tm, m):
                """Q/K production m-tile for macro tm into qkT/K01/K2."""
                tsl = slice(512 * tm, 512 * tm + 512)
                ps = pps.tile([128, 1536], F32, tag="S")
                for dc in range(NDC):
                    nc.tensor.matmul(
                        ps[:, 0:512],
                        wqk_sb[:, dc, 128 * m : 128 * m + 128],
                        xT_sb[:, dc, tsl],
                        start=(dc == 0),
                        stop=(not with_bias and dc == NDC - 1),
                    )
                if with_bias:
                    nc.tensor.matmul(
                        ps[:, 0:512],
                        bqk_sb[0:1, 128 * m : 128 * m + 128],
                        ones_sb[0:1, 0:512],
                        start=False,
                        stop=True,
                    )
                nc.vector.tensor_copy(qkT[m][:, tsl], ps[:, 0:512])
                if m == 1:
                    nc.sync.dma_start(K01[0:64, tsl], qkT[1][64:128, tsl])
                elif m == 2:
                    nc.sync.dma_start(K01[64:128, tsl], qkT[2][0:64, tsl])
                    nc.sync.dma_start(K2[0:64, tsl], qkT[2][64:128, tsl])

            def emit_v_tile(tm, ti):
                """V production t-tile (natural layout) for macro tm."""
                tt = 4 * tm + ti
                psv = pps.tile([128, 1536], F32, tag="S")
                for dc in range(NDC):
                    nc.tensor.matmul(
                        psv[:, 0:192],
                        xT_sb[:, dc, 128 * tt : 128 * tt + 128],
                        wv_sb[:, dc, :],
                        start=(dc == 0),
                        stop=(not with_bias and dc == NDC - 1),
                    )
                if with_bias:
                    nc.tensor.matmul(
                        psv[:, 0:192],
                        ones_sb[0:1, 0:128],
                        bv_sb[0:1, :],
                        start=False,
                        stop=True,
                    )
                nc.vector.tensor_copy(
                    V_sb[:, tt, :].rearrange("p (h c) -> p h c", c=65)[
                        :, :, 0:64
                    ],
                    psv[:, 0:192].rearrange("p (h c) -> p h c", c=64),
                )

            def emit_transposes(qm):
                """Transpose macro qm's normalized attention rows into the
                projection-stationary layout attnT_sb."""
                slot = qm % 2
                psT = pps.tile([128, 3072], BF, tag="S")
                for tt2 in range(4):
                    nc.tensor.transpose(
                        psT[0:128, 128 * tt2 : 128 * tt2 + 128],
                        attn_sb[:, slot, tt2, 0:128],
                        ident_sb[:],
                    )
                    nc.tensor.transpose(
                        psT[0:64, 512 + 128 * tt2 : 640 + 128 * tt2],
                        attn_sb[:, slot, tt2, 128:192],
                        ident_sb[:],
                    )
                nc.vector.tensor_copy(attnT_sb[:, slot, 0, :], psT[:, 0:512])
                nc.vector.tensor_copy(
                    attnT_sb[0:64, slot, 1, :], psT[0:64, 512:1024]
                )

            def emit_proj_tile(qm, tt2, fire_rs):
                """Project 128 rows (macro qm, row-tile tt2) through this
                core's Wproj rows; stage bf16 partials to the RS buffer."""
                slot = qm % 2
                ps = pps.tile([128, 1536], F32, tag="S")
                for half, (c0, c1) in enumerate(((0, 512), (512, 768))):
                    nc.tensor.matmul(
                        ps[:, c0:c1],
                        attnT_sb[:, slot, 0, 128 * tt2 : 128 * tt2 + 128],
                        wp_sb[:, 0, c0:c1],
                        start=True,
                        stop=False,
                    )
                    nc.tensor.matmul(
                        ps[:, c0:c1],
                        attnT_sb[0:65, slot, 1, 128 * tt2 : 128 * tt2 + 128],
                        wp_sb[0:65, 1, c0:c1],
                        start=False,
                        stop=True,
                    )
                stg = wpool.tile([128, D], BF, name="stg", tag="stg")
                nc.vector.tensor_copy(stg[:], ps[:, 0:768])
                j, (qm0, _) = next(
                    (jj, sg) for jj, sg in enumerate(RS_SEGS)
                    if sg[0] <= qm < sg[0] + sg[1]
                )
                r0 = 512 * (qm - qm0) + 128 * tt2
                nc.sync.dma_start(rs_in[j][r0 : r0 + 128, :], stg[:])
                if fire_rs:
                    nc.gpsimd.collective_compute(
                        "ReduceScatter",
                        mybir.AluOpType.add,
                        ins=[rs_in[j][:]],
                        outs=[rs_out[j][:]],
                        replica_groups=[[0, 1, 2, 3], [4, 5, 6, 7]],
                    )

            def emit_finalize(qm):
                """Row sums -> reciprocal -> normalized attn rows (Pool)."""
                O = O_tiles[qm]
                sums = spool.tile([128, 12], F32, tag="sums")
                rsum = spool.tile([128, 12], F32, tag="rsum")
                nc.vector.tensor_copy(
                    sums[:, 0:7],
                    O[:, 64 : 64 + 65 * 7].rearrange(
                        "p (m c) -> p m c", c=65
                    )[:, :, 0:1],
                )
                nc.vector.tensor_copy(
                    sums[:, 7:12],
                    O[:, 512 + 64 : 512 + 64 + 65 * 5].rearrange(
                        "p (m c) -> p m c", c=65
                    )[:, :, 0:1],
                )
                nc.vector.reciprocal(rsum[:], sums[:])
                slot = qm % 2
                # qs-major so the next step's transposes (tt-major) can
                # start after the first few normalize ops
                for qs in range(4):
                    for h in range(3):
                        m_ = 4 * h + qs
                        c0 = _ocol(m_)
                        nc.vector.tensor_scalar_mul(
                            attn_sb[:, slot, qs, 64 * h : 64 * h + 64],
                            O[:, c0 : c0 + 64],
                            rsum[:, m_ : m_ + 1],
                        )

            # ---- prologue: QKV for macros 0 and 1 (the loop produces
            # macro tm+2 during step tm so the K-slice DMAs have two
            # steps of slack against collective-transfer congestion) ----
            nc.sync.dma_start(xT_sb[:, :, 512:1024], xT_v[:, :, 512:1024])
            for m in range(3):
                emit_qk_mtile(0, m)
            for ti in range(4):
                emit_v_tile(0, ti)
            nc.sync.dma_start(xT_sb[:, :, 1024:1536], xT_v[:, :, 1024:1536])
            for m in range(3):
                emit_qk_mtile(1, m)
            for ti in range(4):
                emit_v_tile(1, ti)

            O_tiles = {}
            # global PV pipe (lag 2) carried across q-macro boundaries so
            # the ACT engine never drains while a macro finalizes
            pipe = []

            def pop_pv():
                qm_, kc_, P_ = pipe.pop(0)
                if kc_ == 0:
                    # O tile requested lazily: after E(qm_-1) was emitted,
                    # so the pool WAR covers the previous normalize reads
                    O_tiles[qm_] = ppo.tile(
                        [128, 1024], F32, tag="O", name="O"
                    )
                O_ = O_tiles[qm_]
                j0 = max(0, 128 * kc_ - 512 * qm_)
                for h in range(3):
                    for qs in range(j0 // 128, 4):
                        m_ = 4 * h + qs
                        c0 = _ocol(m_)
                        # start=True clears the has_written bits of the
                        # WHOLE psum bank, so only the first matmul per
                        # bank (m 0 / m 7) may carry it; the rest
                        # fresh-write via cleared bits.
                        nc.tensor.matmul(
                            O_[:, c0 : c0 + 65],
                            P_[:, h, 128 * qs : 128 * qs + 128],
                            V_sb[:, kc_, 65 * h : 65 * h + 65],
                            start=(kc_ == 0 and m_ in (0, 7)),
                            stop=(kc_ == 4 * qm_ + qs),
                        )
                if kc_ == 4 * qm_ + 3:
                    emit_finalize(qm_)

            # ---- main loop over q-macros ----
            for tm in range(NTM):
                qm = tm
                if tm + 3 < NTM:
                    tsl2 = slice(512 * (tm + 3), 512 * (tm + 3) + 512)
                    nc.sync.dma_start(xT_sb[:, :, tsl2], xT_v[:, :, tsl2])

                # interleave pieces: qkv of tm+2 first (two-step lead so
                # its K-slice DMAs clear before any collective transfer
                # hogs the DMA queues), then proj of tm-1 (which must
                # wait for the normalize running on DVE at step start).
                pieces = []
                if tm + 2 < NTM:
                    for m in range(3):
                        pieces.append(lambda t=tm + 2, mm=m: emit_qk_mtile(t, mm))
                    pieces.append(lambda t=tm + 2: emit_v_tile(t, 0))
                if tm >= 1:
                    pqm = tm - 1
                    pieces.append(lambda q=pqm: emit_transposes(q))
                    # fire the segment's ReduceScatter after the last
                    # row-tile of the segment's last macro is staged.
                    for tt2 in range(4):
                        last = tt2 == 3 and any(
                            q0 + n - 1 == pqm for (q0, n) in RS_SEGS
                        )
                        pieces.append(
                            lambda q=pqm, t2=tt2, fr=last: emit_proj_tile(
                                q, t2, fr
                            )
                        )
                if tm + 2 < NTM:
                    for ti in range(1, 4):
                        pieces.append(lambda t=tm + 2, tti=ti: emit_v_tile(t, tti))

                npc = 0
                for kc in range(4 * qm + 4):
                    j0 = max(0, 128 * kc - 512 * qm)
                    S = pps.tile([128, 3, 512], F32, tag="S")
                    q0 = 512 * qm + j0
                    q1 = 512 * qm + 512
                    stats = [
                        K01[0:64, 128 * kc : 128 * kc + 128],
                        K01[64:128, 128 * kc : 128 * kc + 128],
                        K2[0:64, 128 * kc : 128 * kc + 128],
                    ]
                    rhss = [
                        qkT[0][0:64, q0:q1],
                        qkT[0][64:128, q0:q1],
                        qkT[1][0:64, q0:q1],
                    ]
                    diag = kc >= 4 * qm
                    for h in range(3):
                        nc.tensor.matmul(
                            S[:, h, j0:512],
                            stats[h],
                            rhss[h],
                            start=True,
                            stop=not diag,
                        )
                    if diag:
                        # add -1e9 upper-triangle on PE: I.T @ maskneg
                        # (after all three S matmuls: a matmul that follows
                        # an accumulation-group switch pays ~100ns restart,
                        # so don't interleave)
                        for h in range(3):
                            nc.tensor.matmul(
                                S[:, h, j0 : j0 + 128],
                                ident_sb[:],
                                mask_sb[:],
                                start=False,
                                stop=True,
                            )
                    P = ppool.tile([128, 3, 512], BF, tag="P")
                    nc.scalar.activation(
                        P[:, :, j0:512], S[:, :, j0:512], EXP, scale=0.125
                    )
                    pipe.append((qm, kc, P))
                    if len(pipe) > 2:
                        pop_pv()
                    # interleave non-attention PE work in pairs of psum
                    # requests so S tiles keep alternating banks (a final
                    # odd piece goes out alone rather than waiting)
                    if kc >= 3 and kc % 2 == 1 and npc < len(pieces):
                        pieces[npc]()
                        npc += 1
                        if npc < len(pieces):
                            pieces[npc]()
                            npc += 1
                while npc < len(pieces):
                    pieces[npc]()
                    npc += 1

            while pipe:
                pop_pv()

            # ---- epilogue: proj of the last macro + final RS ----
            emit_transposes(NTM - 1)
            for tt2 in range(4):
                emit_proj_tile(NTM - 1, tt2, tt2 == 3)

            # bounce rs_out -> out, all in the epilogue on the sync queue:
            # collectives 0..n-2 have long completed (no stall), and the
            # last one's wait is the inherent tail.
            for j, (_, n) in enumerate(RS_SEGS):
                bnc = wpool.tile([128, 4, D], BF, name="bnc", tag="bnc")
                nc.sync.dma_start(
                    bnc[:, 0:n, :],
                    rs_out[j][:].rearrange("(t p) c -> p t c", p=128),
                )
                o0 = RS_OFFS[j]
                nc.sync.dma_start(
                    out[o0 : o0 + 128 * n, :].rearrange(
                        "(t p) c -> p t c", p=128
                    ),
                    bnc[:, 0:n, :],
                )

    legalize_waits(nc)
    return nc


def _prep_inputs(x, Wqkv, bqkv, Wproj, bproj):
    bf = ml_dtypes.bfloat16
    x = np.asarray(x, np.float32)
    Wqkv = np.asarray(Wqkv, np.float32)
    bqkv = np.asarray(bqkv, np.float32)
    Wproj = np.asarray(Wproj, np.float32)
    bproj = np.asarray(bproj, np.float32)

    # Wqkv columns: head h occupies cols [192h, 192h+192) = [q(64) k(64) v(64)]
    Wh = Wqkv.reshape(D, H, 3, DH)
    bh = bqkv.reshape(H, 3, DH)

    mask = np.where(
        np.arange(128)[None, :] >= np.arange(128)[:, None], 0.0, -1e9
    ).astype(bf)

    in_maps = []
    for c in range(8):
        b, g = c // GROUPS, c % GROUPS
        hs = [NH * g + i for i in range(NH)]
        wqk_ = np.concatenate(
            [Wh[:, h, 0, :] for h in hs] + [Wh[:, h, 1, :] for h in hs], axis=1
        ).astype(bf)
        wv_ = np.concatenate([Wh[:, h, 2, :] for h in hs], axis=1).astype(bf)
        bqk_ = np.concatenate(
            [bh[h, 0, :] for h in hs] + [bh[h, 1, :] for h in hs]
        ).astype(bf)[None, :]
        bvv = np.concatenate([bh[h, 2, :] for h in hs]).astype(bf)[None, :]
        # wp: chunk 0 = Wproj rows [192g, 192g+128); chunk 1 rows 0:64 =
        # Wproj rows [192g+128, 192g+192), row 64 = bproj/4 (ones-row
        # bias trick; the 4 cores of a batch group each add a quarter).
        wp_ = np.zeros((128, 2, D), bf)
        wp_[:, 0, :] = Wproj[192 * g : 192 * g + 128].astype(bf)
        wp_[0:64, 1, :] = Wproj[192 * g + 128 : 192 * g + 192].astype(bf)
        wp_[64, 1, :] = (bproj / 4.0).astype(bf)
        in_maps.append(
            {
                "xT": np.ascontiguousarray(x[b].T).astype(bf),
                "wqk": wqk_,
                "wv": wv_,
                "bqk": bqk_,
                "bv": bvv,
                "wp": wp_,
                "maskp": mask,
            }
        )
    return in_maps


LAST_EXEC_NS = None
LAST_RESULT = None


def kernel(x, Wqkv, bqkv, Wproj, bproj, trace=False):
    global LAST_EXEC_NS, LAST_RESULT
    if trace:
        _install_ntff_hook()
    # bqkv folds in only via explicit matmuls; skip them when it is zero
    # (bproj always rides the wp ones-row for free)
    wb = bool(np.any(np.asarray(bqkv)))
    key = f"nc{wb}"
    if key not in _CACHE:
        _CACHE[key] = _build(with_bias=wb)
    nc = _CACHE[key]
    in_maps = _prep_inputs(x, Wqkv, bqkv, Wproj, bproj)
    try:
        res = run_bass_kernel_spmd(nc, in_maps, list(range(8)), trace=trace)
    except ModuleNotFoundError:
        res = run_bass_kernel_spmd(nc, in_maps, list(range(8)), trace=False)
    LAST_EXEC_NS = res.exec_time_ns
    LAST_RESULT = res
    full = np.zeros((B, T, D), np.float32)
    for c in range(8):
        b, g = c // GROUPS, c % GROUPS
        o = np.asarray(res.results[c]["out"]).astype(np.float32)
        for j, (qm0, n) in enumerate(RS_SEGS):
            o0 = RS_OFFS[j]
            nrows = 128 * n
            r0 = 512 * qm0 + nrows * g
            full[b, r0 : r0 + nrows, :] = o[o0 : o0 + nrows]
    return full


# revision 38
# speedup vs baseline: 1.1025x; 1.1025x over previous
"""Distributed causal multi-head attention kernel for 8 TRN2 NeuronCores.

Sharding: 8 cores = 2 (batch) x 4 (head groups of 3 heads each).
Per core: qkv projection for its 3 heads (bf16 matmuls, f32 accum),
flash-style causal attention entirely in SBUF (S^T layout, no max
subtraction -- logits are bounded ~8 for this distribution), then the
output projection contribution of this core's 192 channels, summed
across the 4 cores of each batch group with per-q-macro-pair
ReduceScatters that overlap the attention loop.

Software pipeline per 512-row q-macro step tm:
  - attention for q-macro tm (S matmuls -> EXP on ACT engine -> PV),
  - QKV production for macro tm+1 and projection of macro tm-1 are
    emitted in pairs between attention kc blocks so the PE fills the
    gaps while the ACT engine runs EXPs,
  - PSUM: 2x[128,1536] rotating S tiles + 1x[128,1024] O tile = 8 banks;
    interleaved pieces borrow S-ring slots in pairs to keep the S tiles
    alternating banks.
"""

import os
import sys
import types
import ctypes
import contextlib

sys.path.insert(0, "/opt/trn_rl_repo")

import numpy as np
import ml_dtypes

import concourse.bass as bass
import concourse.mybir as mybir
import concourse.tile as tile
from concourse.masks import make_identity
from concourse import bass_utils
from concourse.bass_utils import run_bass_kernel_spmd


def _install_ntff_hook():
    """Provide antenv.axon_hooks + the ctypes NTFF profile hook so
    run_bass_kernel_spmd(trace=True) can capture HW exec times under
    axon. No-op if already present or the .so lacks the symbols."""
    try:
        from antenv.axon_hooks import get_axon_ntff_profile_hook  # noqa

        return
    except ImportError:
        pass
    try:
        import antenv
    except ImportError:
        antenv = types.ModuleType("antenv")
        sys.modules["antenv"] = antenv
    mod = types.ModuleType("antenv.axon_hooks")
    mod._hook = None
    mod.set_axon_ntff_profile_hook = lambda h: setattr(mod, "_hook", h)
    mod.get_axon_ntff_profile_hook = lambda: mod._hook
    sys.modules["antenv.axon_hooks"] = mod
    antenv.axon_hooks = mod

    so_path = "/opt/axon/libaxon_pjrt.so"
    if not os.path.exists(so_path):
        return
    try:
        lib = ctypes.CDLL(so_path)
    except OSError:
        return
    if not hasattr(lib, "axon_start_nrt_profile"):
        return
    lib.axon_start_nrt_profile.argtypes = [
        ctypes.POINTER(ctypes.c_int64),
        ctypes.c_size_t,
    ]
    lib.axon_start_nrt_profile.restype = ctypes.c_int64
    lib.axon_stop_nrt_profile.argtypes = [ctypes.c_char_p]
    lib.axon_stop_nrt_profile.restype = ctypes.c_int64

    @contextlib.contextmanager
    def _hook(output_dir, device_ids):
        import jax

        jax.devices()
        if device_ids:
            ids = (ctypes.c_int64 * len(device_ids))(*device_ids)
            rc = lib.axon_start_nrt_profile(ids, len(device_ids))
        else:
            rc = lib.axon_start_nrt_profile(None, 0)
        if rc != 0:
            raise RuntimeError(f"axon_start_nrt_profile rc={rc}")
        try:
            yield
        finally:
            n = lib.axon_stop_nrt_profile(str(output_dir).encode())
            print(f"ntff profile: {n} file(s) written to {output_dir}")

    mod._hook = _hook


# Artifact upload needs a remote bucket; keep everything local instead.
bass_utils.upload_artifacts = lambda tmpdir: str(tmpdir)

dt = mybir.dt
BF = dt.bfloat16
F32 = dt.float32

B, T, D, H, DH = 2, 4096, 768, 12, 64
NH = 3            # heads per core
GROUPS = 4        # head groups (tensor-parallel)
NDC = D // 128    # 6 contraction chunks
NTM = T // 512    # 8 t-macros
NTT = T // 128    # 32 t-tiles
CW = NH * DH      # 192 channels per core

# ReduceScatter segments: (first qm, #qms). The last segment is a
# single macro so the final (non-overlappable) collective is small;
# the others are kept small too so each transfer's DMA-bandwidth
# contention with the compute loop stays short.
RS_SEGS = [(0, 3), (3, 2), (5, 2), (7, 1)]
# output-row offset of each segment's slice in the per-core `out`
RS_OFFS = [0]
for _, _n in RS_SEGS[:-1]:
    RS_OFFS.append(RS_OFFS[-1] + 128 * _n)

_CACHE = {}


def _ocol(m):
    # O-block m (m = 4*h + qs) at col 65*m, with a bank-boundary fix:
    # blocks 0-6 in PSUM bank 0 ([0,512)), blocks 7-11 in bank 1.
    return 65 * m if m < 7 else 512 + 65 * (m - 7)


def legalize_waits(nc):
    """Walrus in this toolchain accepts at most one sync-wait per
    instruction (and none on collectives); hoist excess waits onto
    preceding same-engine NoOps."""
    wi = 0
    for f in nc.m.functions:
        for bb in f.blocks:
            new_insts = []
            changed = False
            for ins in bb.instructions:
                si = ins.sync_info
                if si is None or not si.on_wait:
                    new_insts.append(ins)
                    continue
                merged = {}
                for w in si.on_wait:
                    key = (w.sync_type, w.id, w.wait_mode, str(w.wait_reg))
                    if key not in merged or (w.wait_value or 0) > (
                        merged[key].wait_value or 0
                    ):
                        merged[key] = w
                waits = list(merged.values())
                cap = 0 if isinstance(ins, mybir.InstCollectiveCompute) else 1
                if len(waits) <= cap and len(waits) == len(si.on_wait):
                    new_insts.append(ins)
                    continue
                n_hoist = max(0, len(waits) - cap)
                hoist, keep = waits[:n_hoist], waits[n_hoist:]
                for w in hoist:
                    wi += 1
                    nop = mybir.InstNoOp(name=f"lgw_{wi}", engine=ins.engine)
                    nop.sync_info = mybir.SyncInfo(on_wait=[w], on_update=[])
                    new_insts.append(nop)
                    changed = True
                ins.sync_info = mybir.SyncInfo(
                    on_wait=keep, on_update=list(si.on_update)
                )
                new_insts.append(ins)
            if changed:
                bb.instructions = new_insts


def _build(with_bias=True):
    nc = bass.Bass()
    xT = nc.declare_dram_parameter("xT", [D, T], BF, isOutput=False)
    wqk = nc.declare_dram_parameter("wqk", [D, 2 * CW], BF, isOutput=False)
    wv = nc.declare_dram_parameter("wv", [D, CW], BF, isOutput=False)
    bqk = nc.declare_dram_parameter("bqk", [1, 2 * CW], BF, isOutput=False)
    bv = nc.declare_dram_parameter("bv", [1, CW], BF, isOutput=False)
    wp = nc.declare_dram_parameter("wp", [128, 2, D], BF, isOutput=False)
    maskp = nc.declare_dram_parameter("maskp", [128, 128], BF, isOutput=False)
    out = nc.declare_dram_parameter("out", [1024, D], BF, isOutput=True)

    rs_in = [
        nc.dram_tensor(f"rs_in{j}", [512 * n, D], BF)
        for j, (_, n) in enumerate(RS_SEGS)
    ]
    rs_out = [
        nc.dram_tensor(f"rs_out{j}", [128 * n, D], BF)
        for j, (_, n) in enumerate(RS_SEGS)
    ]
    EXP = mybir.ActivationFunctionType.Exp

    with tile.TileContext(nc) as tc:
        with (
            tc.tile_pool(name="const", bufs=1) as cpool,
            tc.tile_pool(name="work", bufs=3) as wpool,
            tc.tile_pool(name="pwork", bufs=6) as ppool,
            # staging ring deep enough to absorb a whole collective
            # transfer window (~25us) of rs_in DMAs queued behind the
            # CC traffic without jamming the PSUM S-ring via WAR chains
            tc.tile_pool(name="stage", bufs=6) as stpool,
            tc.tile_pool(name="small", bufs=2) as spool,
            tc.tile_pool(name="psS", bufs=2, space="PSUM") as pps,
            tc.tile_pool(name="psO", bufs=1, space="PSUM") as ppo,
        ):
            wqk_sb = cpool.tile([128, NDC, 2 * CW], BF)
            wv_sb = cpool.tile([128, NDC, CW], BF)
            wp_sb = cpool.tile([128, 2, D], BF)
            bqk_sb = cpool.tile([1, 2 * CW], BF)
            bv_sb = cpool.tile([1, CW], BF)
            mask_sb = cpool.tile([128, 128], BF)
            ident_sb = cpool.tile([128, 128], BF)
            ones_sb = cpool.tile([1, 512], BF)
            qkT = [
                cpool.tile([128, T], BF, name=f"qkT{m}", tag=f"qkT{m}")
                for m in range(3)
            ]
            K01 = cpool.tile([128, T], BF)   # rows 0:64 = k0, 64:128 = k1
            K2 = cpool.tile([64, T], BF)     # rows 0:64 = k2
            V_sb = cpool.tile([128, NTT, 3 * 65], BF)
            xT_sb = cpool.tile([128, NDC, T], BF)
            # normalized attention rows for q-macro tm live in slot tm%2
            attn_sb = cpool.tile([128, 2, 4, CW], BF)
            # transposed attention (proj stationary): [part, slot, chunk, t]
            # partition 64 of chunk 1 is a constant 1.0 row (bias trick).
            attnT_sb = cpool.tile([128, 2, 2, 512], BF)

            xT_v = xT[:].rearrange("(dc p) t -> p dc t", p=128)
            wqk_v = wqk[:].rearrange("(dc p) c -> p dc c", p=128)
            wv_v = wv[:].rearrange("(dc p) c -> p dc c", p=128)

            # ---- prologue: first-needed data first, split for parallel
            # DMA queues: B(0) needs xT chunk 0 + wqk (+ bqk), C(0) adds wv.
            for dc in range(NDC):
                nc.sync.dma_start(xT_sb[:, dc, 0:512], xT_v[:, dc, 0:512])
                nc.sync.dma_start(wqk_sb[:, dc, :], wqk_v[:, dc, :])
            if with_bias:
                nc.sync.dma_start(bqk_sb[:], bqk[:])
                nc.sync.dma_start(bv_sb[:], bv[:])
            for dc in range(NDC):
                nc.sync.dma_start(wv_sb[:, dc, :], wv_v[:, dc, :])
            nc.sync.dma_start(mask_sb[:], maskp[:])
            make_identity(nc, ident_sb[:])
            nc.gpsimd.memset(ones_sb[:], 1.0)
            for h in range(3):
                nc.gpsimd.memset(V_sb[:, :, 64 + 65 * h : 65 + 65 * h], 1.0)
            nc.gpsimd.memset(attnT_sb[64:65, :, 1, :], 1.0)
            nc.sync.dma_start(wp_sb[:], wp[:])

            # ---- piece emitters ----
            def emit_qk_mtile(tm, m):
                """Q/K production m-tile for macro tm into qkT/K01/K2."""
                tsl = slice(512 * tm, 512 * tm + 512)
                ps = pps.tile([128, 1536], F32, tag="S")
                for dc in range(NDC):
                    nc.tensor.matmul(
                        ps[:, 0:512],
                        wqk_sb[:, dc, 128 * m : 128 * m + 128],
                        xT_sb[:, dc, tsl],
                        start=(dc == 0),
                        stop=(not with_bias and dc == NDC - 1),
                    )
                if with_bias:
                    nc.tensor.matmul(
                        ps[:, 0:512],
                        bqk_sb[0:1, 128 * m : 128 * m + 128],
                        ones_sb[0:1, 0:512],
                        start=False,
                        stop=True,
                    )
                nc.vector.tensor_copy(qkT[m][:, tsl], ps[:, 0:512])
                if m == 1:
                    nc.sync.dma_start(K01[0:64, tsl], qkT[1][64:128, tsl])
                elif m == 2:
                    nc.sync.dma_start(K01[64:128, tsl], qkT[2][0:64, tsl])
                    nc.sync.dma_start(K2[0:64, tsl], qkT[2][64:128, tsl])

            def emit_v_tile(tm, ti):
                """V production t-tile (natural layout) for macro tm."""
                tt = 4 * tm + ti
                psv = pps.tile([128, 1536], F32, tag="S")
                for dc in range(NDC):
                    nc.tensor.matmul(
                        psv[:, 0:192],
                        xT_sb[:, dc, 128 * tt : 128 * tt + 128],
                        wv_sb[:, dc, :],
                        start=(dc == 0),
                        stop=(not with_bias and dc == NDC - 1),
                    )
                if with_bias:
                    nc.tensor.matmul(
                        psv[:, 0:192],
                        ones_sb[0:1, 0:128],
                        bv_sb[0:1, :],
                        start=False,
                        stop=True,
                    )
                nc.vector.tensor_copy(
                    V_sb[:, tt, :].rearrange("p (h c) -> p h c", c=65)[
                        :, :, 0:64
                    ],
                    psv[:, 0:192].rearrange("p (h c) -> p h c", c=64),
                )

            def emit_transposes(qm):
                """Transpose macro qm's normalized attention rows into the
                projection-stationary layout attnT_sb."""
                slot = qm % 2
                psT = pps.tile([128, 3072], BF, tag="S")
                for tt2 in range(4):
                    nc.tensor.transpose(
                        psT[0:128, 128 * tt2 : 128 * tt2 + 128],
                        attn_sb[:, slot, tt2, 0:128],
                        ident_sb[:],
                    )
                    nc.tensor.transpose(
                        psT[0:64, 512 + 128 * tt2 : 640 + 128 * tt2],
                        attn_sb[:, slot, tt2, 128:192],
                        ident_sb[:],
                    )
                nc.vector.tensor_copy(attnT_sb[:, slot, 0, :], psT[:, 0:512])
                nc.vector.tensor_copy(
                    attnT_sb[0:64, slot, 1, :], psT[0:64, 512:1024]
                )

            def emit_proj_tile(qm, tt2, fire_rs):
                """Project 128 rows (macro qm, row-tile tt2) through this
                core's Wproj rows; stage bf16 partials to the RS buffer."""
                slot = qm % 2
                ps = pps.tile([128, 1536], F32, tag="S")
                for half, (c0, c1) in enumerate(((0, 512), (512, 768))):
                    nc.tensor.matmul(
                        ps[:, c0:c1],
                        attnT_sb[:, slot, 0, 128 * tt2 : 128 * tt2 + 128],
                        wp_sb[:, 0, c0:c1],
                        start=True,
                        stop=False,
                    )
                    nc.tensor.matmul(
                        ps[:, c0:c1],
                        attnT_sb[0:65, slot, 1, 128 * tt2 : 128 * tt2 + 128],
                        wp_sb[0:65, 1, c0:c1],
                        start=False,
                        stop=True,
                    )
                stg = stpool.tile([128, D], BF, name="stg", tag="stg")
                nc.vector.tensor_copy(stg[:], ps[:, 0:768])
                j, (qm0, _) = next(
                    (jj, sg) for jj, sg in enumerate(RS_SEGS)
                    if sg[0] <= qm < sg[0] + sg[1]
                )
                r0 = 512 * (qm - qm0) + 128 * tt2
                nc.sync.dma_start(rs_in[j][r0 : r0 + 128, :], stg[:])
                if fire_rs:
                    nc.gpsimd.collective_compute(
                        "ReduceScatter",
                        mybir.AluOpType.add,
                        ins=[rs_in[j][:]],
                        outs=[rs_out[j][:]],
                        replica_groups=[[0, 1, 2, 3], [4, 5, 6, 7]],
                    )

            def emit_finalize(qm):
                """Row sums -> reciprocal -> normalized attn rows (Pool)."""
                O = O_tiles[qm]
                sums = spool.tile([128, 12], F32, tag="sums")
                rsum = spool.tile([128, 12], F32, tag="rsum")
                nc.vector.tensor_copy(
                    sums[:, 0:7],
                    O[:, 64 : 64 + 65 * 7].rearrange(
                        "p (m c) -> p m c", c=65
                    )[:, :, 0:1],
                )
                nc.vector.tensor_copy(
                    sums[:, 7:12],
                    O[:, 512 + 64 : 512 + 64 + 65 * 5].rearrange(
                        "p (m c) -> p m c", c=65
                    )[:, :, 0:1],
                )
                nc.vector.reciprocal(rsum[:], sums[:])
                slot = qm % 2
                # qs-major so the next step's transposes (tt-major) can
                # start after the first few normalize ops
                for qs in range(4):
                    for h in range(3):
                        m_ = 4 * h + qs
                        c0 = _ocol(m_)
                        nc.vector.tensor_scalar_mul(
                            attn_sb[:, slot, qs, 64 * h : 64 * h + 64],
                            O[:, c0 : c0 + 64],
                            rsum[:, m_ : m_ + 1],
                        )

            # ---- prologue QKV for macro 0, prefetch chunk 1 ----
            for m in range(3):
                emit_qk_mtile(0, m)
            for ti in range(4):
                emit_v_tile(0, ti)
            nc.sync.dma_start(xT_sb[:, :, 512:1024], xT_v[:, :, 512:1024])

            O_tiles = {}
            # global PV pipe (lag 2) carried across q-macro boundaries so
            # the ACT engine never drains while a macro finalizes
            pipe = []

            def pop_pv():
                qm_, kc_, P_ = pipe.pop(0)
                if kc_ == 0:
                    # O tile requested lazily: after E(qm_-1) was emitted,
                    # so the pool WAR covers the previous normalize reads
                    O_tiles[qm_] = ppo.tile(
                        [128, 1024], F32, tag="O", name="O"
                    )
                O_ = O_tiles[qm_]
                j0 = max(0, 128 * kc_ - 512 * qm_)
                for h in range(3):
                    for qs in range(j0 // 128, 4):
                        m_ = 4 * h + qs
                        c0 = _ocol(m_)
                        # start=True clears the has_written bits of the
                        # WHOLE psum bank, so only the first matmul per
                        # bank (m 0 / m 7) may carry it; the rest
                        # fresh-write via cleared bits.
                        nc.tensor.matmul(
                            O_[:, c0 : c0 + 65],
                            P_[:, h, 128 * qs : 128 * qs + 128],
                            V_sb[:, kc_, 65 * h : 65 * h + 65],
                            start=(kc_ == 0 and m_ in (0, 7)),
                            stop=(kc_ == 4 * qm_ + qs),
                        )
                if kc_ == 4 * qm_ + 3:
                    emit_finalize(qm_)

            # ---- main loop over q-macros ----
            for tm in range(NTM):
                qm = tm
                if tm + 2 < NTM:
                    tsl2 = slice(512 * (tm + 2), 512 * (tm + 2) + 512)
                    nc.sync.dma_start(xT_sb[:, :, tsl2], xT_v[:, :, tsl2])

                # interleave pieces: qkv of tm+1 first (its inputs are
                # ready; the proj of tm-1 must wait for the normalize
                # running on DVE at step start), then proj of tm-1.
                pieces = []
                if tm + 1 < NTM:
                    for m in range(3):
                        pieces.append(lambda t=tm + 1, mm=m: emit_qk_mtile(t, mm))
                    pieces.append(lambda t=tm + 1: emit_v_tile(t, 0))
                if tm >= 1:
                    pqm = tm - 1
                    pieces.append(lambda q=pqm: emit_transposes(q))
                    # fire the segment's ReduceScatter after the last
                    # row-tile of the segment's last macro is staged.
                    for tt2 in range(4):
                        last = tt2 == 3 and any(
                            q0 + n - 1 == pqm for (q0, n) in RS_SEGS
                        )
                        pieces.append(
                            lambda q=pqm, t2=tt2, fr=last: emit_proj_tile(
                                q, t2, fr
                            )
                        )
                if tm + 1 < NTM:
                    for ti in range(1, 4):
                        pieces.append(lambda t=tm + 1, tti=ti: emit_v_tile(t, tti))

                npc = 0
                for kc in range(4 * qm + 4):
                    j0 = max(0, 128 * kc - 512 * qm)
                    S = pps.tile([128, 3, 512], F32, tag="S")
                    q0 = 512 * qm + j0
                    q1 = 512 * qm + 512
                    stats = [
                        K01[0:64, 128 * kc : 128 * kc + 128],
                        K01[64:128, 128 * kc : 128 * kc + 128],
                        K2[0:64, 128 * kc : 128 * kc + 128],
                    ]
                    rhss = [
                        qkT[0][0:64, q0:q1],
                        qkT[0][64:128, q0:q1],
                        qkT[1][0:64, q0:q1],
                    ]
                    diag = kc >= 4 * qm
                    for h in range(3):
                        nc.tensor.matmul(
                            S[:, h, j0:512],
                            stats[h],
                            rhss[h],
                            start=True,
                            stop=not diag,
                        )
                    if diag:
                        # add -1e9 upper-triangle on PE: I.T @ maskneg
                        # (after all three S matmuls: a matmul that follows
                        # an accumulation-group switch pays ~100ns restart,
                        # so don't interleave)
                        for h in range(3):
                            nc.tensor.matmul(
                                S[:, h, j0 : j0 + 128],
                                ident_sb[:],
                                mask_sb[:],
                                start=False,
                                stop=True,
                            )
                    P = ppool.tile([128, 3, 512], BF, tag="P")
                    nc.scalar.activation(
                        P[:, :, j0:512], S[:, :, j0:512], EXP, scale=0.125
                    )
                    pipe.append((qm, kc, P))
                    if len(pipe) > 2:
                        pop_pv()
                    # interleave non-attention PE work in pairs of psum
                    # requests so S tiles keep alternating banks (a final
                    # odd piece goes out alone rather than waiting)
                    if kc >= 3 and kc % 2 == 1 and npc < len(pieces):
                        pieces[npc]()
                        npc += 1
                        if npc < len(pieces):
                            pieces[npc]()
                            npc += 1
                while npc < len(pieces):
                    pieces[npc]()
                    npc += 1

            while pipe:
                pop_pv()

            # ---- epilogue: proj of the last macro + final RS ----
            emit_transposes(NTM - 1)
            for tt2 in range(4):
                emit_proj_tile(NTM - 1, tt2, tt2 == 3)

            # bounce rs_out -> out, all in the epilogue on the sync queue:
            # collectives 0..n-2 have long completed (no stall), and the
            # last one's wait is the inherent tail.
            for j, (_, n) in enumerate(RS_SEGS):
                bnc = wpool.tile([128, 4, D], BF, name="bnc", tag="bnc")
                nc.sync.dma_start(
                    bnc[:, 0:n, :],
                    rs_out[j][:].rearrange("(t p) c -> p t c", p=128),
                )
                o0 = RS_OFFS[j]
                nc.sync.dma_start(
                    out[o0 : o0 + 128 * n, :].rearrange(
                        "(t p) c -> p t c", p=128
                    ),
                    bnc[:, 0:n, :],
                )

    legalize_waits(nc)
    return nc


def _prep_inputs(x, Wqkv, bqkv, Wproj, bproj):
    bf = ml_dtypes.bfloat16
    x = np.asarray(x, np.float32)
    Wqkv = np.asarray(Wqkv, np.float32)
    bqkv = np.asarray(bqkv, np.float32)
    Wproj = np.asarray(Wproj, np.float32)
    bproj = np.asarray(bproj, np.float32)

    # Wqkv columns: head h occupies cols [192h, 192h+192) = [q(64) k(64) v(64)]
    Wh = Wqkv.reshape(D, H, 3, DH)
    bh = bqkv.reshape(H, 3, DH)

    mask = np.where(
        np.arange(128)[None, :] >= np.arange(128)[:, None], 0.0, -1e9
    ).astype(bf)

    in_maps = []
    for c in range(8):
        b, g = c // GROUPS, c % GROUPS
        hs = [NH * g + i for i in range(NH)]
        wqk_ = np.concatenate(
            [Wh[:, h, 0, :] for h in hs] + [Wh[:, h, 1, :] for h in hs], axis=1
        ).astype(bf)
        wv_ = np.concatenate([Wh[:, h, 2, :] for h in hs], axis=1).astype(bf)
        bqk_ = np.concatenate(
            [bh[h, 0, :] for h in hs] + [bh[h, 1, :] for h in hs]
        ).astype(bf)[None, :]
        bvv = np.concatenate([bh[h, 2, :] for h in hs]).astype(bf)[None, :]
        # wp: chunk 0 = Wproj rows [192g, 192g+128); chunk 1 rows 0:64 =
        # Wproj rows [192g+128, 192g+192), row 64 = bproj/4 (ones-row
        # bias trick; the 4 cores of a batch group each add a quarter).
        wp_ = np.zeros((128, 2, D), bf)
        wp_[:, 0, :] = Wproj[192 * g : 192 * g + 128].astype(bf)
        wp_[0:64, 1, :] = Wproj[192 * g + 128 : 192 * g + 192].astype(bf)
        wp_[64, 1, :] = (bproj / 4.0).astype(bf)
        in_maps.append(
            {
                "xT": np.ascontiguousarray(x[b].T).astype(bf),
                "wqk": wqk_,
                "wv": wv_,
                "bqk": bqk_,
                "bv": bvv,
                "wp": wp_,
                "maskp": mask,
            }
        )
    return in_maps


LAST_EXEC_NS = None
LAST_RESULT = None


def kernel(x, Wqkv, bqkv, Wproj, bproj, trace=False):
    global LAST_EXEC_NS, LAST_RESULT
    if trace:
        _install_ntff_hook()
    # bqkv folds in only via explicit matmuls; skip them when it is zero
    # (bproj always rides the wp ones-row for free)
    wb = bool(np.any(np.asarray(bqkv)))
    key = f"nc{wb}"
    if key not in _CACHE:
        _CACHE[key] = _build(with_bias=wb)
    nc = _CACHE[key]
    in_maps = _prep_inputs(x, Wqkv, bqkv, Wproj, bproj)
    try:
        res = run_bass_kernel_spmd(nc, in_maps, list(range(8)), trace=trace)
    except ModuleNotFoundError:
        res = run_bass_kernel_spmd(nc, in_maps, list(range(8)), trace=False)
    LAST_EXEC_NS = res.exec_time_ns
    LAST_RESULT = res
    full = np.zeros((B, T, D), np.float32)
    for c in range(8):
        b, g = c // GROUPS, c % GROUPS
        o = np.asarray(res.results[c]["out"]).astype(np.float32)
        for j, (qm0, n) in enumerate(RS_SEGS):
            o0 = RS_OFFS[j]
            nrows = 128 * n
            r0 = 512 * qm0 + nrows * g
            full[b, r0 : r0 + nrows, :] = o[o0 : o0 + nrows]
    return full
